# revision 11
# baseline (speedup 1.0000x reference)
"""Linformer self-attention (degenerate-einsum variant) on 8 TRN2 NeuronCores.

Math (from the reference):
  k_proj[b,h,k,d] = E[k,d] * S_k[b,h*64+d]  where S_k[b,:] = (sum_n x[b,n,:]) @ Wk.T
  (the einsum 'bhnd,kd->bhkd' sums k over n, elementwise in d; the sequence sum
   commutes with the linear projection, so k/v never need materializing)
  attn = softmax( (q * S_k) @ E.T / 8 )  per (b, head)
  out  = (attn @ (F * S_v)) restored to (B,N,D), then @ Wo.T + bo

Sharding: core c = (batch b = c//2, sequence half = c%2); each core computes a
(2048, 1024) slice of the output. Host precomputes S_k/S_v (tiny) and folds
them into per-head E-hat (fp32r) and F-hat (bf16, block-diagonal pair packing),
pre-transposes x / Wq / Wo, and pre-rounds fp32r operands.
"""

import numpy as np
import ml_dtypes

import concourse.bass as bass
import concourse.bacc as bacc
import concourse.tile as tile
import concourse.mybir as mybir
import concourse.bass_utils as bass_utils

B, N, D = 4, 4096, 1024
H, HD, KP = 16, 64, 256  # heads, head dim, linformer K
NCORES = 8
NH = N // 2          # rows per core = 2048
HBLK = 256           # half-block rows
NHB = NH // HBLK     # 8 half-blocks
F32 = mybir.dt.float32
F32R = mybir.dt.float32r
BF16 = mybir.dt.bfloat16

_CACHE = {}


def _round_fp32r(a: np.ndarray) -> np.ndarray:
    """Round-to-nearest-even fp32 -> fp32r (11 explicit mantissa bits)."""
    b = np.ascontiguousarray(a, dtype=np.float32).view(np.uint32)
    low = b & np.uint32(0xFFF)
    bit12 = (b >> np.uint32(12)) & np.uint32(1)
    up = (low > 0x800) | ((low == 0x800) & (bit12 == 1))
    r = (b & np.uint32(0xFFFFF000)) + (up.astype(np.uint32) << np.uint32(12))
    return r.view(np.float32)


def _build():
    nc = bacc.Bacc("TRN2", target_bir_lowering=False, debug=False, num_devices=NCORES)

    xT_d = nc.dram_tensor("xT", [D, NH], F32R, kind="ExternalInput").ap()
    wqT_d = nc.dram_tensor("wqT", [D, D], F32R, kind="ExternalInput").ap()
    woT_d = nc.dram_tensor("woT", [D, D], F32R, kind="ExternalInput").ap()
    ehat_d = nc.dram_tensor("ehat", [128, 8, 2 * KP], F32R, kind="ExternalInput").ap()
    fhat_d = nc.dram_tensor("fhat", [128, 8, 2, 2, 128], BF16, kind="ExternalInput").ap()
    bo_d = nc.dram_tensor("bo", [1, D], F32R, kind="ExternalInput").ap()
    ident_d = nc.dram_tensor("ident", [128, 128], BF16, kind="ExternalInput").ap()
    ones_d = nc.dram_tensor("ones", [1, 128], F32R, kind="ExternalInput").ap()
    out_d = nc.dram_tensor("out", [NH, D], F32, kind="ExternalOutput").ap()

    with tile.TileContext(nc) as tc:
        with (
            tc.tile_pool(name="wq", bufs=1) as wq_pool,
            tc.tile_pool(name="wo", bufs=1) as wo_pool,
            tc.tile_pool(name="const", bufs=1) as const_pool,
            tc.tile_pool(name="xt", bufs=12) as xt_pool,
            tc.tile_pool(name="qt", bufs=15) as qt_pool,
            tc.tile_pool(name="estat", bufs=8) as stat_pool,
            tc.tile_pool(name="ep", bufs=8) as e_pool,
            tc.tile_pool(name="pt", bufs=33) as pt_pool,
            tc.tile_pool(name="ohat", bufs=12) as ohat_pool,
            tc.tile_pool(name="osb", bufs=3) as out_pool,
            tc.tile_pool(name="qfpsum", bufs=2, space=bass.MemorySpace.PSUM) as qfpsum,
            tc.tile_pool(name="apsum", bufs=3, space=bass.MemorySpace.PSUM) as apsum,
            tc.tile_pool(name="ppsum", bufs=2, space=bass.MemorySpace.PSUM) as ppsum,
            tc.tile_pool(name="opsum", bufs=1, space=bass.MemorySpace.PSUM) as opsum,
        ):
            # ---- persistent weights ----
            wq_sb = []
            wo_sb = []
            for c in range(8):
                t = wq_pool.tile([128, D], F32R, tag=f"wq{c}")
                nc.sync.dma_start(t[:], wqT_d[c * 128:(c + 1) * 128, :])
                wq_sb.append(t)
                t = wo_pool.tile([128, D], F32R, tag=f"wo{c}")
                nc.sync.dma_start(t[:], woT_d[c * 128:(c + 1) * 128, :])
                wo_sb.append(t)
            ehat_sb = const_pool.tile([128, 8, 2 * KP], F32R, tag="ehat")
            nc.sync.dma_start(ehat_sb[:], ehat_d[:])
            fhat_sb = const_pool.tile([128, 8, 2, 2, 128], BF16, tag="fhat")
            nc.sync.dma_start(fhat_sb[:], fhat_d[:])
            bo_sb = const_pool.tile([1, D], F32R, tag="bo")
            nc.sync.dma_start(bo_sb[:], bo_d[:])
            ident_sb = const_pool.tile([128, 128], BF16, tag="ident")
            nc.sync.dma_start(ident_sb[:], ident_d[:])
            ones_sb = const_pool.tile([1, 128], F32R, tag="ones")
            nc.sync.dma_start(ones_sb[:], ones_d[:])

            # ---- software-pipelined main loop over half-blocks of 256 rows ----
            # stage A(hb): DMA xT, Q-proj, attn logits + softmax -> p tiles
            # stage B(hb): transposes, ohat, final, store — emitted one hb late
            # so the PE never waits on freshly-computed softmax results.
            p_state = {}

            def stage_a(hb):
                blk = hb // 2
                if hb % 2 == 0:
                    xt = []
                    for c in range(8):
                        t = xt_pool.tile([128, 512], F32R, tag="xt", name=f"xt{c}")
                        nc.sync.dma_start(
                            t[:], xT_d[c * 128:(c + 1) * 128, blk * 512:(blk + 1) * 512]
                        )
                        xt.append(t)
                    qt = []
                    for co in range(8):
                        qp = qfpsum.tile([128, 512], F32, tag="qf", name=f"qp{co}")
                        for ck in range(8):
                            nc.tensor.matmul(
                                qp[:],
                                wq_sb[ck][:, co * 128:(co + 1) * 128],
                                xt[ck][:],
                                start=(ck == 0),
                                stop=(ck == 7),
                            )
                        q_sb = qt_pool.tile([128, 512], F32R, tag="qt", name=f"q{co}")
                        nc.scalar.copy(q_sb[:], qp[:])
                        qt.append(q_sb)
                    p_state[(blk, "qt")] = qt
                qt = p_state[(blk, "qt")]

                pts = []
                for h in range(H):
                    pts.append(pt_pool.tile([128, 2, HBLK], BF16, tag="pt", name=f"pt{h}"))
                for s in range(2):
                    sb = (hb % 2) * 2 + s
                    for g in range(4):  # groups of 2 pairs = 4 heads
                        aps = []
                        negmax = stat_pool.tile([128, 4], F32, tag="negmax")
                        ssum = stat_pool.tile([128, 4], F32, tag="ssum")
                        for jj in range(2):
                            j = 2 * g + jj
                            ap_ = apsum.tile([128, 2 * KP], F32, tag="ap", name=f"ap{j}")
                            nc.tensor.matmul(
                                ap_[:],
                                qt[j][:, sb * 128:(sb + 1) * 128],
                                ehat_sb[:, j, :],
                                start=True,
                                stop=True,
                            )
                            aps.append(ap_)
                            nc.vector.reduce_max(
                                negmax[:, 2 * jj:2 * jj + 2],
                                ap_[:].rearrange("p (c k) -> p c k", c=2),
                                axis=mybir.AxisListType.X, negate=True,
                            )
                        e_tiles = []
                        for hh in range(4):
                            h = 4 * g + hh
                            e_sb = e_pool.tile([128, KP], BF16, tag="e", name=f"e{h}")
                            nc.scalar.activation(
                                e_sb[:],
                                aps[hh // 2][:, (hh % 2) * KP:(hh % 2 + 1) * KP],
                                mybir.ActivationFunctionType.Exp,
                                bias=negmax[:, hh:hh + 1], accum_out=ssum[:, hh:hh + 1],
                            )
                            e_tiles.append(e_sb)
                        recip = stat_pool.tile([128, 4], F32, tag="recip")
                        nc.vector.reciprocal(recip[:], ssum[:])
                        for hh in range(4):
                            h = 4 * g + hh
                            p_sb = e_pool.tile([128, KP], BF16, tag="p", name=f"p{h}")
                            nc.gpsimd.tensor_scalar_mul(
                                p_sb[:], e_tiles[hh][:], recip[:, hh:hh + 1]
                            )
                            p_state[(hb, s, h)] = p_sb
                p_state[(hb, "pts")] = pts

            def stage_b(hb):
                r0 = hb * HBLK
                pts = p_state.pop((hb, "pts"))
                for s in range(2):
                    for h in range(H):
                        p_sb = p_state.pop((hb, s, h))
                        ptp = ppsum.tile([128, KP], BF16, tag="ptp", name=f"ptp{h}")
                        for c in range(2):
                            nc.tensor.transpose(
                                ptp[:, c * 128:(c + 1) * 128],
                                p_sb[:, c * 128:(c + 1) * 128],
                                ident_sb[:],
                            )
                        eng_copy = nc.scalar.copy if h % 2 == 0 else nc.vector.tensor_copy
                        eng_copy(
                            pts[h][:, :, s * 128:(s + 1) * 128],
                            ptp[:].rearrange("p (c r) -> p c r", c=2),
                        )
                ohatT = []
                for j in range(8):
                    op_ = opsum.tile([128, HBLK], F32, tag="op", name=f"op{j}")
                    first = True
                    for hh in range(2):
                        for c in range(2):
                            nc.tensor.matmul(
                                op_[:],
                                fhat_sb[:, j, hh, c, :],
                                pts[2 * j + hh][:, c, :],
                                start=first,
                                stop=(hh == 1 and c == 1),
                            )
                            first = False
                    oT = ohat_pool.tile([128, HBLK], F32R, tag="ohatT", name=f"oT{j}")
                    nc.scalar.copy(oT[:], op_[:])
                    ohatT.append(oT)
                for s in range(2):
                    for half in range(2):
                        fp_ = qfpsum.tile([128, 512], F32, tag="qf", name=f"fp{s}{half}")
                        for j in range(8):
                            nc.tensor.matmul(
                                fp_[:],
                                ohatT[j][:, s * 128:(s + 1) * 128],
                                wo_sb[j][:, half * 512:(half + 1) * 512],
                                start=(j == 0),
                                stop=False,
                            )
                        nc.tensor.matmul(
                            fp_[:],
                            ones_sb[:],
                            bo_sb[0:1, half * 512:(half + 1) * 512],
                            start=False,
                            stop=True,
                        )
                        o_sb = out_pool.tile([128, 512], F32, tag="osb", name=f"o{s}{half}")
                        nc.scalar.copy(o_sb[:], fp_[:])
                        nc.sync.dma_start(
                            out_d[r0 + s * 128:r0 + (s + 1) * 128,
                                  half * 512:(half + 1) * 512],
                            o_sb[:],
                        )

            for hb in range(NHB + 1):
                if hb < NHB:
                    stage_a(hb)
                if hb >= 1:
                    stage_b(hb - 1)
                    if hb % 2 == 0:
                        p_state.pop(((hb - 1) // 2, "qt"), None)

    nc.compile()
    return nc


def _prep_inputs(x, Wq, Wk, Wv, E, F, Wo, bo):
    x = np.asarray(x, dtype=np.float32)
    Wq = np.asarray(Wq, dtype=np.float32)
    Wk = np.asarray(Wk, dtype=np.float32)
    Wv = np.asarray(Wv, dtype=np.float32)
    E = np.asarray(E, dtype=np.float32)
    F_ = np.asarray(F, dtype=np.float32)
    Wo = np.asarray(Wo, dtype=np.float32)
    bo = np.asarray(bo, dtype=np.float32)

    xsum = x.sum(axis=1)                     # (B, D)
    S_k = xsum @ Wk.T                        # (B, D)
    S_v = xsum @ Wv.T                        # (B, D)

    wqT = _round_fp32r(np.ascontiguousarray(Wq.T))
    woT = _round_fp32r(np.ascontiguousarray(Wo.T))
    bo_row = _round_fp32r(bo.reshape(1, D))
    ident = np.eye(128, dtype=ml_dtypes.bfloat16)

    in_maps = []
    for core in range(NCORES):
        b, half = core // 2, core % 2
        xs = x[b, half * NH:(half + 1) * NH, :]          # (NH, D)
        xT = _round_fp32r(np.ascontiguousarray(xs.T))    # (D, NH)

        # E-hat: block-diagonal per head pair -> one (128,512) rhs per pair
        ehat = np.zeros((128, 8, 2 * KP), dtype=np.float32)
        for h in range(H):
            sk = S_k[b, h * HD:(h + 1) * HD]             # (64,)
            j, hh = h // 2, h % 2
            ehat[hh * 64:hh * 64 + 64, j, hh * KP:(hh + 1) * KP] = (E.T * sk[:, None]) / 8.0
        ehat = _round_fp32r(ehat)

        # F-hat: block-diagonal pair packing, (128, pair, head-in-pair, chunk, 64*2)
        fhat = np.zeros((128, 8, 2, 2, 128), dtype=np.float32)
        for h in range(H):
            sv = S_v[b, h * HD:(h + 1) * HD]             # (64,)
            fh = F_ * sv[None, :]                        # (KP, 64)
            j, hh = h // 2, h % 2
            for c in range(2):
                fhat[:, j, hh, c, hh * 64:(hh + 1) * 64] = fh[c * 128:(c + 1) * 128, :]
        fhat = fhat.astype(ml_dtypes.bfloat16)

        in_maps.append({
            "xT": xT, "wqT": wqT, "woT": woT, "ehat": ehat,
            "fhat": fhat, "bo": bo_row, "ident": ident,
            "ones": np.ones((1, 128), dtype=np.float32),
        })
    return in_maps


def _run(inputs: dict, trace: bool = False, tmpdir: str | None = None):
    if "nc" not in _CACHE:
        _CACHE["nc"] = _build()
    nc = _CACHE["nc"]
    in_maps = _prep_inputs(**inputs)
    res = bass_utils.run_bass_kernel_spmd(
        nc, in_maps, core_ids=list(range(NCORES)), trace=trace, tmpdir=tmpdir
    )
    out = np.empty((B, N, D), dtype=np.float32)
    for core in range(NCORES):
        b, half = core // 2, core % 2
        out[b, half * NH:(half + 1) * NH, :] = res.results[core]["out"]
    return out, res


def kernel(**inputs) -> np.ndarray:
    out, _ = _run(inputs)
    return out


# revision 12
# speedup vs baseline: 3.0131x; 3.0131x over previous
"""Linformer self-attention (degenerate-einsum variant) on 8 TRN2 NeuronCores.

Math (from the reference):
  k_proj[b,h,k,d] = E[k,d] * S_k[b,h*64+d]  where S_k[b,:] = (sum_n x[b,n,:]) @ Wk.T
  (the einsum 'bhnd,kd->bhkd' sums k over n, elementwise in d; the sequence sum
   commutes with the linear projection, so k/v never need materializing)
  attn = softmax( (q * S_k) @ E.T / 8 )  per (b, head)
  out  = (attn @ (F * S_v)) restored to (B,N,D), then @ Wo.T + bo

Sharding: core c = (batch b = c//2, sequence half = c%2); each core computes a
(2048, 1024) slice of the output. Host precomputes S_k/S_v (tiny) and folds
them into per-head E-hat (fp32r) and F-hat (bf16, block-diagonal pair packing),
pre-transposes x / Wq / Wo, and pre-rounds fp32r operands.
"""

import numpy as np
import ml_dtypes

import concourse.bass as bass
import concourse.bacc as bacc
import concourse.tile as tile
import concourse.mybir as mybir
import concourse.bass_utils as bass_utils

B, N, D = 4, 4096, 1024
H, HD, KP = 16, 64, 256  # heads, head dim, linformer K
NCORES = 8
NH = N // 2          # rows per core = 2048
HBLK = 256           # half-block rows
NHB = NH // HBLK     # 8 half-blocks
F32 = mybir.dt.float32
F32R = mybir.dt.float32r
BF16 = mybir.dt.bfloat16

_CACHE = {}


def _round_fp32r(a: np.ndarray) -> np.ndarray:
    """Round-to-nearest-even fp32 -> fp32r (11 explicit mantissa bits)."""
    b = np.ascontiguousarray(a, dtype=np.float32).view(np.uint32)
    low = b & np.uint32(0xFFF)
    bit12 = (b >> np.uint32(12)) & np.uint32(1)
    up = (low > 0x800) | ((low == 0x800) & (bit12 == 1))
    r = (b & np.uint32(0xFFFFF000)) + (up.astype(np.uint32) << np.uint32(12))
    return r.view(np.float32)


def _build():
    nc = bacc.Bacc("TRN2", target_bir_lowering=False, debug=False, num_devices=NCORES)

    xT_d = nc.dram_tensor("xT", [D, NH], F32R, kind="ExternalInput").ap()
    wqT_d = nc.dram_tensor("wqT", [D, D], F32R, kind="ExternalInput").ap()
    woT_d = nc.dram_tensor("woT", [D, D], F32R, kind="ExternalInput").ap()
    ehat_d = nc.dram_tensor("ehat", [128, 8, 2 * KP], F32R, kind="ExternalInput").ap()
    fhat_d = nc.dram_tensor("fhat", [128, 8, 2, 2, 128], BF16, kind="ExternalInput").ap()
    bo_d = nc.dram_tensor("bo", [1, D], F32R, kind="ExternalInput").ap()
    ident_d = nc.dram_tensor("ident", [128, 128], BF16, kind="ExternalInput").ap()
    ones_d = nc.dram_tensor("ones", [1, 128], F32R, kind="ExternalInput").ap()
    out_d = nc.dram_tensor("out", [NH, D], F32, kind="ExternalOutput").ap()

    with tile.TileContext(nc) as tc:
        with (
            tc.tile_pool(name="wq", bufs=1) as wq_pool,
            tc.tile_pool(name="wo", bufs=1) as wo_pool,
            tc.tile_pool(name="const", bufs=1) as const_pool,
            tc.tile_pool(name="xt", bufs=12) as xt_pool,
            tc.tile_pool(name="qt", bufs=15) as qt_pool,
            tc.tile_pool(name="estat", bufs=8) as stat_pool,
            tc.tile_pool(name="ep", bufs=8) as e_pool,
            tc.tile_pool(name="pt", bufs=33) as pt_pool,
            tc.tile_pool(name="ohat", bufs=12) as ohat_pool,
            tc.tile_pool(name="osb", bufs=3) as out_pool,
            tc.tile_pool(name="qfpsum", bufs=2, space=bass.MemorySpace.PSUM) as qfpsum,
            tc.tile_pool(name="apsum", bufs=3, space=bass.MemorySpace.PSUM) as apsum,
            tc.tile_pool(name="ppsum", bufs=2, space=bass.MemorySpace.PSUM) as ppsum,
            tc.tile_pool(name="opsum", bufs=1, space=bass.MemorySpace.PSUM) as opsum,
        ):
            # ---- persistent weights ----
            wq_sb = []
            wo_sb = []
            for c in range(8):
                t = wq_pool.tile([128, D], F32R, tag=f"wq{c}")
                nc.sync.dma_start(t[:], wqT_d[c * 128:(c + 1) * 128, :])
                wq_sb.append(t)
                t = wo_pool.tile([128, D], F32R, tag=f"wo{c}")
                nc.sync.dma_start(t[:], woT_d[c * 128:(c + 1) * 128, :])
                wo_sb.append(t)
            ehat_sb = const_pool.tile([128, 8, 2 * KP], F32R, tag="ehat")
            nc.sync.dma_start(ehat_sb[:], ehat_d[:])
            fhat_sb = const_pool.tile([128, 8, 2, 2, 128], BF16, tag="fhat")
            nc.sync.dma_start(fhat_sb[:], fhat_d[:])
            bo_sb = const_pool.tile([1, D], F32R, tag="bo")
            nc.sync.dma_start(bo_sb[:], bo_d[:])
            ident_sb = const_pool.tile([128, 128], BF16, tag="ident")
            nc.sync.dma_start(ident_sb[:], ident_d[:])
            ones_sb = const_pool.tile([1, 128], F32R, tag="ones")
            nc.sync.dma_start(ones_sb[:], ones_d[:])

            # ---- software-pipelined main loop over half-blocks of 256 rows ----
            # stage A(hb): DMA xT, Q-proj, attn logits + softmax -> p tiles
            # stage B(hb): transposes, ohat, final, store — emitted one hb late
            # so the PE never waits on freshly-computed softmax results.
            p_state = {}

            def stage_a(hb):
                blk = hb // 2
                if hb % 2 == 0:
                    xt = []
                    for c in range(8):
                        t = xt_pool.tile([128, 512], F32R, tag="xt", name=f"xt{c}")
                        nc.sync.dma_start(
                            t[:], xT_d[c * 128:(c + 1) * 128, blk * 512:(blk + 1) * 512]
                        )
                        xt.append(t)
                    qt = []
                    for co in range(8):
                        qp = qfpsum.tile([128, 512], F32, tag="qf", name=f"qp{co}")
                        for ck in range(8):
                            nc.tensor.matmul(
                                qp[:],
                                wq_sb[ck][:, co * 128:(co + 1) * 128],
                                xt[ck][:],
                                start=(ck == 0),
                                stop=(ck == 7),
                            )
                        q_sb = qt_pool.tile([128, 512], F32R, tag="qt", name=f"q{co}")
                        nc.scalar.copy(q_sb[:], qp[:])
                        qt.append(q_sb)
                    p_state[(blk, "qt")] = qt
                qt = p_state[(blk, "qt")]

                pts = []
                for h in range(H):
                    pts.append(pt_pool.tile([128, 2, HBLK], BF16, tag="pt", name=f"pt{h}"))
                for s in range(2):
                    sb = (hb % 2) * 2 + s
                    for g in range(4):  # groups of 2 pairs = 4 heads
                        aps = []
                        negmax = stat_pool.tile([128, 4], F32, tag="negmax")
                        ssum = stat_pool.tile([128, 4], F32, tag="ssum")
                        for jj in range(2):
                            j = 2 * g + jj
                            ap_ = apsum.tile([128, 2 * KP], F32, tag="ap", name=f"ap{j}")
                            nc.tensor.matmul(
                                ap_[:],
                                qt[j][:, sb * 128:(sb + 1) * 128],
                                ehat_sb[:, j, :],
                                start=True,
                                stop=True,
                            )
                            aps.append(ap_)
                            nc.vector.reduce_max(
                                negmax[:, 2 * jj:2 * jj + 2],
                                ap_[:].rearrange("p (c k) -> p c k", c=2),
                                axis=mybir.AxisListType.X, negate=True,
                            )
                        e_tiles = []
                        for hh in range(4):
                            h = 4 * g + hh
                            e_sb = e_pool.tile([128, KP], BF16, tag="e", name=f"e{h}")
                            nc.scalar.activation(
                                e_sb[:],
                                aps[hh // 2][:, (hh % 2) * KP:(hh % 2 + 1) * KP],
                                mybir.ActivationFunctionType.Exp,
                                bias=negmax[:, hh:hh + 1], accum_out=ssum[:, hh:hh + 1],
                            )
                            e_tiles.append(e_sb)
                        recip = stat_pool.tile([128, 4], F32, tag="recip")
                        nc.vector.reciprocal(recip[:], ssum[:])
                        for hh in range(4):
                            h = 4 * g + hh
                            p_sb = e_pool.tile([128, KP], BF16, tag="p", name=f"p{h}")
                            nc.vector.tensor_scalar_mul(
                                p_sb[:], e_tiles[hh][:], recip[:, hh:hh + 1]
                            )
                            p_state[(hb, s, h)] = p_sb
                p_state[(hb, "pts")] = pts

            def stage_b(hb):
                r0 = hb * HBLK
                pts = p_state.pop((hb, "pts"))
                for s in range(2):
                    for h in range(H):
                        p_sb = p_state.pop((hb, s, h))
                        ptp = ppsum.tile([128, KP], BF16, tag="ptp", name=f"ptp{h}")
                        for c in range(2):
                            nc.tensor.transpose(
                                ptp[:, c * 128:(c + 1) * 128],
                                p_sb[:, c * 128:(c + 1) * 128],
                                ident_sb[:],
                            )
                        eng_copy = nc.scalar.copy if h % 2 == 0 else nc.vector.tensor_copy
                        eng_copy(
                            pts[h][:, :, s * 128:(s + 1) * 128],
                            ptp[:].rearrange("p (c r) -> p c r", c=2),
                        )
                ohatT = []
                for j in range(8):
                    op_ = opsum.tile([128, HBLK], F32, tag="op", name=f"op{j}")
                    first = True
                    for hh in range(2):
                        for c in range(2):
                            nc.tensor.matmul(
                                op_[:],
                                fhat_sb[:, j, hh, c, :],
                                pts[2 * j + hh][:, c, :],
                                start=first,
                                stop=(hh == 1 and c == 1),
                            )
                            first = False
                    oT = ohat_pool.tile([128, HBLK], F32R, tag="ohatT", name=f"oT{j}")
                    nc.scalar.copy(oT[:], op_[:])
                    ohatT.append(oT)
                for s in range(2):
                    for half in range(2):
                        fp_ = qfpsum.tile([128, 512], F32, tag="qf", name=f"fp{s}{half}")
                        for j in range(8):
                            nc.tensor.matmul(
                                fp_[:],
                                ohatT[j][:, s * 128:(s + 1) * 128],
                                wo_sb[j][:, half * 512:(half + 1) * 512],
                                start=(j == 0),
                                stop=False,
                            )
                        nc.tensor.matmul(
                            fp_[:],
                            ones_sb[:],
                            bo_sb[0:1, half * 512:(half + 1) * 512],
                            start=False,
                            stop=True,
                        )
                        o_sb = out_pool.tile([128, 512], F32, tag="osb", name=f"o{s}{half}")
                        nc.scalar.copy(o_sb[:], fp_[:])
                        nc.sync.dma_start(
                            out_d[r0 + s * 128:r0 + (s + 1) * 128,
                                  half * 512:(half + 1) * 512],
                            o_sb[:],
                        )

            for hb in range(NHB + 1):
                if hb < NHB:
                    stage_a(hb)
                if hb >= 1:
                    stage_b(hb - 1)
                    if hb % 2 == 0:
                        p_state.pop(((hb - 1) // 2, "qt"), None)

    nc.compile()
    return nc


def _prep_inputs(x, Wq, Wk, Wv, E, F, Wo, bo):
    x = np.asarray(x, dtype=np.float32)
    Wq = np.asarray(Wq, dtype=np.float32)
    Wk = np.asarray(Wk, dtype=np.float32)
    Wv = np.asarray(Wv, dtype=np.float32)
    E = np.asarray(E, dtype=np.float32)
    F_ = np.asarray(F, dtype=np.float32)
    Wo = np.asarray(Wo, dtype=np.float32)
    bo = np.asarray(bo, dtype=np.float32)

    xsum = x.sum(axis=1)                     # (B, D)
    S_k = xsum @ Wk.T                        # (B, D)
    S_v = xsum @ Wv.T                        # (B, D)

    wqT = _round_fp32r(np.ascontiguousarray(Wq.T))
    woT = _round_fp32r(np.ascontiguousarray(Wo.T))
    bo_row = _round_fp32r(bo.reshape(1, D))
    ident = np.eye(128, dtype=ml_dtypes.bfloat16)

    in_maps = []
    for core in range(NCORES):
        b, half = core // 2, core % 2
        xs = x[b, half * NH:(half + 1) * NH, :]          # (NH, D)
        xT = _round_fp32r(np.ascontiguousarray(xs.T))    # (D, NH)

        # E-hat: block-diagonal per head pair -> one (128,512) rhs per pair
        ehat = np.zeros((128, 8, 2 * KP), dtype=np.float32)
        for h in range(H):
            sk = S_k[b, h * HD:(h + 1) * HD]             # (64,)
            j, hh = h // 2, h % 2
            ehat[hh * 64:hh * 64 + 64, j, hh * KP:(hh + 1) * KP] = (E.T * sk[:, None]) / 8.0
        ehat = _round_fp32r(ehat)

        # F-hat: block-diagonal pair packing, (128, pair, head-in-pair, chunk, 64*2)
        fhat = np.zeros((128, 8, 2, 2, 128), dtype=np.float32)
        for h in range(H):
            sv = S_v[b, h * HD:(h + 1) * HD]             # (64,)
            fh = F_ * sv[None, :]                        # (KP, 64)
            j, hh = h // 2, h % 2
            for c in range(2):
                fhat[:, j, hh, c, hh * 64:(hh + 1) * 64] = fh[c * 128:(c + 1) * 128, :]
        fhat = fhat.astype(ml_dtypes.bfloat16)

        in_maps.append({
            "xT": xT, "wqT": wqT, "woT": woT, "ehat": ehat,
            "fhat": fhat, "bo": bo_row, "ident": ident,
            "ones": np.ones((1, 128), dtype=np.float32),
        })
    return in_maps


def _run(inputs: dict, trace: bool = False, tmpdir: str | None = None):
    if "nc" not in _CACHE:
        _CACHE["nc"] = _build()
    nc = _CACHE["nc"]
    in_maps = _prep_inputs(**inputs)
    res = bass_utils.run_bass_kernel_spmd(
        nc, in_maps, core_ids=list(range(NCORES)), trace=trace, tmpdir=tmpdir
    )
    out = np.empty((B, N, D), dtype=np.float32)
    for core in range(NCORES):
        b, half = core // 2, core % 2
        out[b, half * NH:(half + 1) * NH, :] = res.results[core]["out"]
    return out, res


def kernel(**inputs) -> np.ndarray:
    out, _ = _run(inputs)
    return out


# revision 13
# speedup vs baseline: 3.2819x; 1.0892x over previous
"""Linformer self-attention (degenerate-einsum variant) on 8 TRN2 NeuronCores.

Math (from the reference):
  k_proj[b,h,k,d] = E[k,d] * S_k[b,h*64+d]  where S_k[b,:] = (sum_n x[b,n,:]) @ Wk.T
  (the einsum 'bhnd,kd->bhkd' sums k over n, elementwise in d; the sequence sum
   commutes with the linear projection, so k/v never need materializing)
  attn = softmax( (q * S_k) @ E.T / 8 )  per (b, head)
  out  = (attn @ (F * S_v)) restored to (B,N,D), then @ Wo.T + bo

Sharding: core c = (batch b = c//2, sequence half = c%2); each core computes a
(2048, 1024) slice of the output. Host precomputes S_k/S_v (tiny) and folds
them into per-head E-hat (fp32r) and F-hat (bf16, block-diagonal pair packing),
pre-transposes x / Wq / Wo, and pre-rounds fp32r operands.
"""

import numpy as np
import ml_dtypes

import concourse.bass as bass
import concourse.bacc as bacc
import concourse.tile as tile
import concourse.mybir as mybir
import concourse.bass_utils as bass_utils

B, N, D = 4, 4096, 1024
H, HD, KP = 16, 64, 256  # heads, head dim, linformer K
NCORES = 8
NH = N // 2          # rows per core = 2048
HBLK = 256           # half-block rows
NHB = NH // HBLK     # 8 half-blocks
F32 = mybir.dt.float32
F32R = mybir.dt.float32r
BF16 = mybir.dt.bfloat16

_CACHE = {}


def _round_fp32r(a: np.ndarray) -> np.ndarray:
    """Round-to-nearest-even fp32 -> fp32r (11 explicit mantissa bits)."""
    b = np.ascontiguousarray(a, dtype=np.float32).view(np.uint32)
    low = b & np.uint32(0xFFF)
    bit12 = (b >> np.uint32(12)) & np.uint32(1)
    up = (low > 0x800) | ((low == 0x800) & (bit12 == 1))
    r = (b & np.uint32(0xFFFFF000)) + (up.astype(np.uint32) << np.uint32(12))
    return r.view(np.float32)


def _build():
    nc = bacc.Bacc("TRN2", target_bir_lowering=False, debug=False, num_devices=NCORES)

    xT_d = nc.dram_tensor("xT", [D, NH], F32R, kind="ExternalInput").ap()
    wqT_d = nc.dram_tensor("wqT", [D, D], F32R, kind="ExternalInput").ap()
    woT_d = nc.dram_tensor("woT", [D, D], F32R, kind="ExternalInput").ap()
    ehat_d = nc.dram_tensor("ehat", [128, 8, 2 * KP], F32R, kind="ExternalInput").ap()
    fhat_d = nc.dram_tensor("fhat", [128, 8, 2, 2, 128], BF16, kind="ExternalInput").ap()
    bo_d = nc.dram_tensor("bo", [1, D], F32R, kind="ExternalInput").ap()
    ident_d = nc.dram_tensor("ident", [128, 128], BF16, kind="ExternalInput").ap()
    ones_d = nc.dram_tensor("ones", [1, 128], F32R, kind="ExternalInput").ap()
    out_d = nc.dram_tensor("out", [NH, D], F32, kind="ExternalOutput").ap()

    with tile.TileContext(nc) as tc:
        with (
            tc.tile_pool(name="wq", bufs=1) as wq_pool,
            tc.tile_pool(name="wo", bufs=1) as wo_pool,
            tc.tile_pool(name="const", bufs=1) as const_pool,
            tc.tile_pool(name="xt", bufs=10) as xt_pool,
            tc.tile_pool(name="qt", bufs=14) as qt_pool,
            tc.tile_pool(name="estat", bufs=8) as stat_pool,
            tc.tile_pool(name="ep", bufs=8) as e_pool,
            tc.tile_pool(name="pp", bufs=40) as p_pool,
            tc.tile_pool(name="pt", bufs=20) as pt_pool,
            tc.tile_pool(name="ohat", bufs=10) as ohat_pool,
            tc.tile_pool(name="osb", bufs=3) as out_pool,
            tc.tile_pool(name="qfpsum", bufs=2, space=bass.MemorySpace.PSUM) as qfpsum,
            tc.tile_pool(name="apsum", bufs=3, space=bass.MemorySpace.PSUM) as apsum,
            tc.tile_pool(name="ppsum", bufs=2, space=bass.MemorySpace.PSUM) as ppsum,
            tc.tile_pool(name="opsum", bufs=1, space=bass.MemorySpace.PSUM) as opsum,
        ):
            # ---- persistent weights ----
            wq_sb = []
            wo_sb = []
            for c in range(8):
                t = wq_pool.tile([128, D], F32R, tag=f"wq{c}")
                nc.sync.dma_start(t[:], wqT_d[c * 128:(c + 1) * 128, :])
                wq_sb.append(t)
                t = wo_pool.tile([128, D], F32R, tag=f"wo{c}")
                nc.sync.dma_start(t[:], woT_d[c * 128:(c + 1) * 128, :])
                wo_sb.append(t)
            ehat_sb = const_pool.tile([128, 8, 2 * KP], F32R, tag="ehat")
            nc.sync.dma_start(ehat_sb[:], ehat_d[:])
            fhat_sb = const_pool.tile([128, 8, 2, 2, 128], BF16, tag="fhat")
            nc.sync.dma_start(fhat_sb[:], fhat_d[:])
            bo_sb = const_pool.tile([1, D], F32R, tag="bo")
            nc.sync.dma_start(bo_sb[:], bo_d[:])
            ident_sb = const_pool.tile([128, 128], BF16, tag="ident")
            nc.sync.dma_start(ident_sb[:], ident_d[:])
            ones_sb = const_pool.tile([1, 128], F32R, tag="ones")
            nc.sync.dma_start(ones_sb[:], ones_d[:])

            # ---- software-pipelined main loop over half-blocks of 256 rows ----
            # stage A(hb): DMA xT, Q-proj, attn logits + softmax -> p tiles
            # stage B(hb): transposes, ohat, final, store — emitted one hb late
            # so the PE never waits on freshly-computed softmax results.
            p_state = {}

            def stage_a(hb):
                blk = hb // 2
                if hb % 2 == 0:
                    xt = []
                    for c in range(8):
                        t = xt_pool.tile([128, 512], F32R, tag="xt", name=f"xt{c}")
                        nc.sync.dma_start(
                            t[:], xT_d[c * 128:(c + 1) * 128, blk * 512:(blk + 1) * 512]
                        )
                        xt.append(t)
                    qt = []
                    for co in range(8):
                        qp = qfpsum.tile([128, 512], F32, tag="qf", name=f"qp{co}")
                        for ck in range(8):
                            nc.tensor.matmul(
                                qp[:],
                                wq_sb[ck][:, co * 128:(co + 1) * 128],
                                xt[ck][:],
                                start=(ck == 0),
                                stop=(ck == 7),
                            )
                        q_sb = qt_pool.tile([128, 512], F32R, tag="qt", name=f"q{co}")
                        nc.scalar.copy(q_sb[:], qp[:])
                        qt.append(q_sb)
                    p_state[(blk, "qt")] = qt
                qt = p_state[(blk, "qt")]

                for s in range(2):
                    sb = (hb % 2) * 2 + s
                    for g in range(4):  # groups of 2 pairs = 4 heads
                        aps = []
                        negmax = stat_pool.tile([128, 4], F32, tag="negmax")
                        ssum = stat_pool.tile([128, 4], F32, tag="ssum")
                        for jj in range(2):
                            j = 2 * g + jj
                            ap_ = apsum.tile([128, 2 * KP], F32, tag="ap", name=f"ap{j}")
                            nc.tensor.matmul(
                                ap_[:],
                                qt[j][:, sb * 128:(sb + 1) * 128],
                                ehat_sb[:, j, :],
                                start=True,
                                stop=True,
                            )
                            aps.append(ap_)
                            nc.vector.reduce_max(
                                negmax[:, 2 * jj:2 * jj + 2],
                                ap_[:].rearrange("p (c k) -> p c k", c=2),
                                axis=mybir.AxisListType.X, negate=True,
                            )
                        e_tiles = []
                        for hh in range(4):
                            h = 4 * g + hh
                            e_sb = e_pool.tile([128, KP], BF16, tag="e", name=f"e{h}")
                            nc.scalar.activation(
                                e_sb[:],
                                aps[hh // 2][:, (hh % 2) * KP:(hh % 2 + 1) * KP],
                                mybir.ActivationFunctionType.Exp,
                                bias=negmax[:, hh:hh + 1], accum_out=ssum[:, hh:hh + 1],
                            )
                            e_tiles.append(e_sb)
                        recip = stat_pool.tile([128, 4], F32, tag="recip")
                        nc.vector.reciprocal(recip[:], ssum[:])
                        for hh in range(4):
                            h = 4 * g + hh
                            p_sb = p_pool.tile([128, KP], BF16, tag="p", name=f"p{h}")
                            nc.vector.tensor_scalar_mul(
                                p_sb[:], e_tiles[hh][:], recip[:, hh:hh + 1]
                            )
                            p_state[(hb, s, h)] = p_sb

            def stage_b(hb):
                r0 = hb * HBLK
                pts = []
                for h in range(H):
                    pts.append(pt_pool.tile([128, 2, HBLK], BF16, tag="pt", name=f"pt{h}"))
                for s in range(2):
                    for h in range(H):
                        p_sb = p_state.pop((hb, s, h))
                        ptp = ppsum.tile([128, KP], BF16, tag="ptp", name=f"ptp{h}")
                        for c in range(2):
                            nc.tensor.transpose(
                                ptp[:, c * 128:(c + 1) * 128],
                                p_sb[:, c * 128:(c + 1) * 128],
                                ident_sb[:],
                            )
                        eng_copy = nc.scalar.copy if h % 2 == 0 else nc.vector.tensor_copy
                        eng_copy(
                            pts[h][:, :, s * 128:(s + 1) * 128],
                            ptp[:].rearrange("p (c r) -> p c r", c=2),
                        )
                ohatT = []
                for j in range(8):
                    op_ = opsum.tile([128, HBLK], F32, tag="op", name=f"op{j}")
                    first = True
                    for hh in range(2):
                        for c in range(2):
                            nc.tensor.matmul(
                                op_[:],
                                fhat_sb[:, j, hh, c, :],
                                pts[2 * j + hh][:, c, :],
                                start=first,
                                stop=(hh == 1 and c == 1),
                            )
                            first = False
                    oT = ohat_pool.tile([128, HBLK], F32R, tag="ohatT", name=f"oT{j}")
                    nc.vector.tensor_copy(oT[:], op_[:])
                    ohatT.append(oT)
                for s in range(2):
                    for half in range(2):
                        fp_ = qfpsum.tile([128, 512], F32, tag="qf", name=f"fp{s}{half}")
                        for j in range(8):
                            nc.tensor.matmul(
                                fp_[:],
                                ohatT[j][:, s * 128:(s + 1) * 128],
                                wo_sb[j][:, half * 512:(half + 1) * 512],
                                start=(j == 0),
                                stop=False,
                            )
                        nc.tensor.matmul(
                            fp_[:],
                            ones_sb[:],
                            bo_sb[0:1, half * 512:(half + 1) * 512],
                            start=False,
                            stop=True,
                        )
                        o_sb = out_pool.tile([128, 512], F32, tag="osb", name=f"o{s}{half}")
                        nc.scalar.copy(o_sb[:], fp_[:])
                        nc.sync.dma_start(
                            out_d[r0 + s * 128:r0 + (s + 1) * 128,
                                  half * 512:(half + 1) * 512],
                            o_sb[:],
                        )

            for hb in range(NHB + 1):
                if hb < NHB:
                    stage_a(hb)
                if hb >= 1:
                    stage_b(hb - 1)
                    if hb % 2 == 0:
                        p_state.pop(((hb - 1) // 2, "qt"), None)

    nc.compile()
    return nc


def _prep_inputs(x, Wq, Wk, Wv, E, F, Wo, bo):
    x = np.asarray(x, dtype=np.float32)
    Wq = np.asarray(Wq, dtype=np.float32)
    Wk = np.asarray(Wk, dtype=np.float32)
    Wv = np.asarray(Wv, dtype=np.float32)
    E = np.asarray(E, dtype=np.float32)
    F_ = np.asarray(F, dtype=np.float32)
    Wo = np.asarray(Wo, dtype=np.float32)
    bo = np.asarray(bo, dtype=np.float32)

    xsum = x.sum(axis=1)                     # (B, D)
    S_k = xsum @ Wk.T                        # (B, D)
    S_v = xsum @ Wv.T                        # (B, D)

    wqT = _round_fp32r(np.ascontiguousarray(Wq.T))
    woT = _round_fp32r(np.ascontiguousarray(Wo.T))
    bo_row = _round_fp32r(bo.reshape(1, D))
    ident = np.eye(128, dtype=ml_dtypes.bfloat16)

    in_maps = []
    for core in range(NCORES):
        b, half = core // 2, core % 2
        xs = x[b, half * NH:(half + 1) * NH, :]          # (NH, D)
        xT = _round_fp32r(np.ascontiguousarray(xs.T))    # (D, NH)

        # E-hat: block-diagonal per head pair -> one (128,512) rhs per pair
        ehat = np.zeros((128, 8, 2 * KP), dtype=np.float32)
        for h in range(H):
            sk = S_k[b, h * HD:(h + 1) * HD]             # (64,)
            j, hh = h // 2, h % 2
            ehat[hh * 64:hh * 64 + 64, j, hh * KP:(hh + 1) * KP] = (E.T * sk[:, None]) / 8.0
        ehat = _round_fp32r(ehat)

        # F-hat: block-diagonal pair packing, (128, pair, head-in-pair, chunk, 64*2)
        fhat = np.zeros((128, 8, 2, 2, 128), dtype=np.float32)
        for h in range(H):
            sv = S_v[b, h * HD:(h + 1) * HD]             # (64,)
            fh = F_ * sv[None, :]                        # (KP, 64)
            j, hh = h // 2, h % 2
            for c in range(2):
                fhat[:, j, hh, c, hh * 64:(hh + 1) * 64] = fh[c * 128:(c + 1) * 128, :]
        fhat = fhat.astype(ml_dtypes.bfloat16)

        in_maps.append({
            "xT": xT, "wqT": wqT, "woT": woT, "ehat": ehat,
            "fhat": fhat, "bo": bo_row, "ident": ident,
            "ones": np.ones((1, 128), dtype=np.float32),
        })
    return in_maps


def _run(inputs: dict, trace: bool = False, tmpdir: str | None = None):
    if "nc" not in _CACHE:
        _CACHE["nc"] = _build()
    nc = _CACHE["nc"]
    in_maps = _prep_inputs(**inputs)
    res = bass_utils.run_bass_kernel_spmd(
        nc, in_maps, core_ids=list(range(NCORES)), trace=trace, tmpdir=tmpdir
    )
    out = np.empty((B, N, D), dtype=np.float32)
    for core in range(NCORES):
        b, half = core // 2, core % 2
        out[b, half * NH:(half + 1) * NH, :] = res.results[core]["out"]
    return out, res


def kernel(**inputs) -> np.ndarray:
    out, _ = _run(inputs)
    return out
